# revision 21
# baseline (speedup 1.0000x reference)
"""Trainium2 Bass kernel for CompositionalPhoneticsModel (segment_reduce).

Computation (reference):
    phone   = einsum('bth,hp->btp', enc_output, feature2phone) / sqrt(H)
    allo    = where(mapping>0, phone[:,:,None,:]*mapping, -inf)   # mapping is 0/1
    phoneme = max(allo, axis=-1)                                  # masked segment max
    out     = log_softmax(phoneme, axis=2)

Device strategy (8 NeuronCores, data-parallel over the B*T=8192 rows):
  * Host gathers feature2phone columns into segment-contiguous order
    (phones in 2 segments get duplicated columns; NNZ ~ 506) and sorts
    segments by length so the per-segment max is a handful of strided DVE
    reduce_max ops.  Host un-permutes the output columns at the end.
  * enc and W ship as fp8 e3m4 (range +-15.5, 4 mantissa bits).  Measured
    absmax/scale error 0.0113 vs the 2e-2 gate, matching the numpy
    simulation bit-for-bit - the PE's single-fp8 path keeps all 4 mantissa
    bits.  The 1/sqrt(H) scale is NOT folded into W (it would push W into
    the e3m4 denormal range); it rides the Exp activation's scale input
    and a fused MULT in the final tensor_scalar.  fp8 halves the DMA bytes
    (loads: 0.97MB/core) so the stream never paces the PE.
  * DMA is bandwidth-bound (~220-290 GB/s/core) once packets (one per
    partition per transfer) are >=1KB, with ~2.4us from program start to
    first byte.  The first matmul gates on W + enc block 0, so W goes
    first, then enc in pieces sized so the stream stays ahead of the PE:
    blocks [0], [1], [2-3], [4-7].  Stores: blocks 0-5 after their subs,
    block 6 alone, block 7 split by partition halves across two queues -
    only ~0.5us of store is exposed after the last compute op.
  * Postlude per 256-row megatile: batched strided segment-max reduces
    (DVE), per-128-row Exp on ScalarE with the row-sum from the activation
    accumulator, one Ln, and a fused (x*scale - lse) DVE tensor_scalar.
    The subs are emitted one megatile LATE: they wait ~1us on ScalarE's
    exp/ln chain and the in-order DVE queue would head-of-line block the
    next megatile's reduces.  The last megatile uses one single-bank PSUM
    tile per 128-row block (a shared tile makes the hazard tracker
    serialize block 7's matmuls behind block 6's reduces) and runs its
    postludes per block, so the post-stream tail is one block's chain.
  * PE warmup matmuls are counted to END right as W + enc block 0 land:
    an idle gap resets the tensor engine's p-state ramp and the next ~14
    real matmuls run ~2x slower.
"""

from contextlib import ExitStack

import numpy as np
import ml_dtypes

import concourse.bass as bass
import concourse.bacc as bacc
import concourse.tile as tile
from concourse import mybir
from concourse.bass_utils import run_bass_kernel_spmd

B, T, H = 8, 1024, 640
N_PHONEME, N_PHONE = 96, 230
N_CORES = 8
ROWS = B * T
RC = ROWS // N_CORES          # rows per core
NH = H // 128                 # contraction chunks
NB = RC // 128                # 128-row blocks per core
NMT = NB // 2                 # megatiles (2 blocks each)
BF16 = ml_dtypes.bfloat16
F8 = ml_dtypes.float8_e3m4
SCALE = float(1.0 / np.sqrt(np.float32(H)))


def _structure(mapping: np.ndarray):
    """Segment-contiguous gather order, grouped by segment length (desc).

    Returns (col_ids, groups, perm):
      col_ids: phone index feeding each device matmul column (len NNZ)
      groups:  list of (L, nL, col_off, out_off) — nL segments of length L
               occupy matmul cols [col_off, col_off+nL*L) and device output
               cols [out_off, out_off+nL)
      perm:    perm[j] = original phoneme id of device output column j
    """
    segs = [np.nonzero(mapping[m] > 0)[0] for m in range(N_PHONEME)]
    assert min(len(s) for s in segs) >= 1
    # pad segment lengths up to even targets (repeating a member doesn't
    # change the max): fewer distinct lengths -> fewer DVE reduce ops.
    # Only worthwhile while the matmul width stays within one PSUM bank.
    padded = []
    for s in segs:
        t = ((len(s) + 1) // 2) * 2
        padded.append(np.concatenate([s, np.full(t - len(s), s[0], s.dtype)]))
    if sum(len(s) for s in padded) <= 512:
        segs = padded
    lengths = np.array([len(s) for s in segs])
    order = np.argsort(-lengths, kind="stable")
    col_ids, groups, perm = [], [], []
    i = 0
    while i < N_PHONEME:
        L = int(lengths[order[i]])
        j = i
        while j < N_PHONEME and lengths[order[j]] == L:
            j += 1
        groups.append((L, j - i, len(col_ids), i))
        for k in range(i, j):
            m = int(order[k])
            col_ids.extend(segs[m].tolist())
            perm.append(m)
        i = j
    return np.array(col_ids, dtype=np.int64), groups, np.array(perm, dtype=np.int64)


def _patch_act_tables():
    """Make Exp and Ln resolve to the same activation-table set.

    bacc's insert_act_table_loads models a single table slot, so a kernel
    alternating Exp/Ln reloads a 1.3us table on every transition.  act_info
    has a joint set ('natural_log_exp_and_others') containing both; keep the
    set list's order/indices intact but strip Exp/Ln from the other sets so
    the pass picks the joint set for both and emits a single load.
    """
    if getattr(bacc, "_act_tables_patched", False):
        return
    from concourse import hw_specs
    orig = hw_specs.get_activation_tables
    act = mybir.ActivationFunctionType

    def patched(module_arch):
        tabs = orig(module_arch)
        joint = [k for k, v in tabs.items() if act.Exp in v and act.Ln in v]
        if not joint:
            return tabs
        j = joint[0]
        return {
            k: (v if k == j else (v - {act.Exp, act.Ln}))
            for k, v in tabs.items()
        }

    bacc.get_activation_tables = patched
    bacc._act_tables_patched = True


def _build_program(nnz: int, groups):
    """Build + compile the per-core Bass program. Returns the Bacc object."""
    _patch_act_tables()
    nc = bacc.Bacc("TRN2", target_bir_lowering=False, debug=False)
    dt = mybir.dt
    act = mybir.ActivationFunctionType
    X = mybir.AxisListType.X
    alu = mybir.AluOpType

    # enc interleaved: [128, RC, NH]; element (p, r, c) = enc[r, c*128+p]
    enck_d = nc.dram_tensor("enck", [128, RC, NH], dt.float8e3, kind="ExternalInput")
    # W interleaved: [128, NH, nnz]; element (p, c, n) = W[c*128+p, n], fp8
    wk_d = nc.dram_tensor("wk", [128, NH, nnz], dt.float8e3, kind="ExternalInput")
    # out packed: [128, NB, 96]; element (p, b, m) = out[b*128+p, m]
    out_d = nc.dram_tensor("out", [128, NB, N_PHONEME], dt.float32, kind="ExternalOutput")

    with ExitStack() as ctx:
        tc = ctx.enter_context(tile.TileContext(nc))
        wpool = ctx.enter_context(tc.tile_pool(name="wpool", bufs=1))
        epool = ctx.enter_context(tc.tile_pool(name="epool", bufs=1))
        # 3 double-bank megatile accumulators + 2 single-bank ones for the
        # last megatile (separate tiles per 128-row block there, so its r0
        # postlude reads never alias r1's accumulation in the hazard
        # tracker - a shared tile serializes the PE behind the DVE)
        ppool = ctx.enter_context(tc.tile_pool(name="ppool", bufs=2, space="PSUM"))
        ppool2 = ctx.enter_context(tc.tile_pool(name="ppool2", bufs=1, space="PSUM"))
        ppool3 = ctx.enter_context(tc.tile_pool(name="ppool3", bufs=2, space="PSUM"))
        spool = ctx.enter_context(tc.tile_pool(name="spool", bufs=3))
        opool = ctx.enter_context(tc.tile_pool(name="opool", bufs=1))

        # Loads, strictly ordered on the Sync queue (the stream is
        # bandwidth-bound, so order = priority): W gates the first matmul,
        # then enc pieces sized to stay ahead of the PE.
        wt = wpool.tile([128, NH, nnz], dt.float8e3)
        et = epool.tile([128, RC, NH], dt.float8e3)
        # the first matmul needs only enc block 0 and W chunks 0-1 (chunk
        # c feeds the c-th accumulation step); later chunks/blocks stream
        # in behind while the PE works
        nc.sync.dma_start(et[:, 0:128, :], enck_d[:, 0:128, :])
        nc.sync.dma_start(wt[:, 0:2, :], wk_d[:, 0:2, :])
        nc.sync.dma_start(wt[:, 2:, :], wk_d[:, 2:, :])
        for lo, hi in ((128, 256), (256, 512), (512, RC)):
            nc.sync.dma_start(et[:, lo:hi, :], enck_d[:, lo:hi, :])

        # PE warmup: dummy matmuls ramp the tensor engine's p-state while
        # the DMAs land.  The operand is the preamble-initialized bf16
        # const (broadcast AP) so no memset gates them - they start at
        # engine-ready (~7.4us) instead of behind the Vector engine.
        # They write the first megatile's PSUM bank; the real
        # accumulation overwrites it.  Count: they must END right as the
        # first matmul's data lands (~10us) - an idle gap resets the
        # ramp (the next ~14 matmuls then run at ~2x duration), overshoot
        # delays the real stream behind the warmup queue.
        wub = nc.const_aps.aps[(dt.bfloat16, 1.0)]
        ps0 = ppool.tile([128, 2, 512], dt.float32, tag="ps")
        for _ in range(5):
            nc.tensor.matmul(ps0[:, 0, :], wub.broadcast_to([128, 128]),
                             wub.broadcast_to([128, 512]), start=True, stop=True)

        obuf = opool.tile([128, NB, N_PHONEME], dt.float32)

        def seg_max(ps, rr, nr, tagn):
            """Segment max of PSUM rows `rr`: one strided DVE reduce per
            segment-length group.  (A pairwise tensor_tensor pre-max would
            halve the reduce work, but the DVE can read only ONE operand
            from PSUM per instruction - walrus NCC_IBVF027.)"""
            pmax = spool.tile([128, nr, N_PHONEME], dt.float32, tag=f"pmax{tagn}")
            for (L, nL, coff, ooff) in groups:
                src = ps[:, rr, coff:coff + nL * L].rearrange(
                    "p r (s l) -> p r s l", l=L
                )
                nc.vector.reduce_max(pmax[:, :, ooff:ooff + nL], src, axis=X)
            return pmax

        def postlude_a(ps, rr, blk, nr):
            """Segment max + exp-sum + Ln for `nr` blocks at `blk`; the
            final subs are emitted LATER (postlude_b) so they queue behind
            the next megatile's reduces: a sub waits on this tile's Ln
            (~1us of ScalarE chain), and the DVE executes in order - subs
            emitted eagerly head-of-line block the next reduces."""
            pmax = seg_max(ps, rr, nr, nr if nr == 2 else blk)
            ex = spool.tile([128, nr, N_PHONEME], dt.float32, tag=f"ex{blk}")
            se = spool.tile([128, nr], dt.float32, tag=f"se{blk}")
            for k in range(nr):
                # exp(scale*x); the row-sum comes free via the activation
                # accumulator (1/sqrt(H) lives here, not in fp8 W)
                nc.scalar.activation(ex[:, k, :], pmax[:, k, :], act.Exp,
                                     scale=SCALE, accum_out=se[:, k:k + 1])
            lse = spool.tile([128, nr], dt.float32, tag=f"lse{blk}")
            nc.scalar.activation(lse[:], se[:], act.Ln)
            return pmax, lse

        def postlude_b(state, blk, nr):
            pmax, lse = state
            for k in range(nr):
                # out = scale*pmax - lse, fused in one DVE op
                nc.vector.tensor_scalar(
                    obuf[:, blk + k, :], pmax[:, k, :],
                    SCALE, lse[:, k:k + 1], op0=alu.mult, op1=alu.subtract,
                )

        def block_matmuls(ps_row, blk, lo=0, hi=None):
            hi = nnz if hi is None else hi
            row0 = blk * 128
            for c in range(NH):
                nc.tensor.matmul(
                    ps_row[:, :hi - lo],
                    et[:, row0:row0 + 128, c],
                    wt[:, c, lo:hi],
                    start=(c == 0),
                    stop=(c == NH - 1),
                )

        states = []
        for mt in range(NMT - 1):
            ps = ps0 if mt == 0 else ppool.tile([128, 2, 512], dt.float32, tag="ps")
            for r in range(2):
                block_matmuls(ps[:, r, :], mt * 2 + r)
            states.append(postlude_a(ps, slice(0, 2), mt * 2, 2))
            if mt > 0:
                postlude_b(states[mt - 1], (mt - 1) * 2, 2)
        # last megatile: one single-bank tile per block so neither block's
        # postlude aliases the other's accumulation in the hazard tracker
        psr0 = ppool2.tile([128, 1, 512], dt.float32, tag="psr")
        block_matmuls(psr0[:, 0, :], NB - 2)
        st6 = postlude_a(psr0, slice(0, 1), NB - 2, 1)
        # block 7 accumulates in two column halves in SEPARATE banks, so
        # the first half's segment-max reduces overlap the second half's
        # matmuls (same-tile reads/writes would falsely serialize in the
        # hazard tracker) - the post-stream reduce work drops to the
        # second half's two groups.
        csplit_i = next(i for i, g in enumerate(groups)
                        if g[2] + g[0] * g[1] > nnz * 3 // 5)
        csplit = groups[csplit_i][2] + groups[csplit_i][0] * groups[csplit_i][1]
        ga, gb = groups[:csplit_i + 1], groups[csplit_i + 1:]
        ps7a = ppool3.tile([128, 1, 512], dt.float32, tag="ps7")
        ps7b = ppool3.tile([128, 1, 512], dt.float32, tag="ps7")
        pm7 = spool.tile([128, 1, N_PHONEME], dt.float32, tag="pmax7")
        block_matmuls(ps7a[:, 0, :], NB - 1, 0, csplit)
        for (L, nL, coff, ooff) in ga:
            nc.vector.reduce_max(
                pm7[:, :, ooff:ooff + nL],
                ps7a[:, 0:1, coff:coff + nL * L].rearrange("p r (s l) -> p r s l", l=L),
                axis=X)
        block_matmuls(ps7b[:, 0, :], NB - 1, csplit, nnz)
        for (L, nL, coff, ooff) in gb:
            nc.vector.reduce_max(
                pm7[:, :, ooff:ooff + nL],
                ps7b[:, 0:1, coff - csplit:coff - csplit + nL * L].rearrange(
                    "p r (s l) -> p r s l", l=L),
                axis=X)
        ex7 = spool.tile([128, 1, N_PHONEME], dt.float32, tag="ex7")
        se7 = spool.tile([128, 1], dt.float32, tag="se7")
        nc.scalar.activation(ex7[:, 0, :], pm7[:, 0, :], act.Exp,
                             scale=SCALE, accum_out=se7[:, 0:1])
        lse7 = spool.tile([128, 1], dt.float32, tag="lse7")
        nc.scalar.activation(lse7[:], se7[:], act.Ln)
        st7 = (pm7, lse7)
        # MT2's subs are deferred past BOTH tail-block reduce sets: on the
        # in-order DVE queue anything emitted between A6 and A7 delays
        # block 7's chain, and the blocks-0-5 store they feed is not the
        # critical DMA (the block-6/7 stores are).
        postlude_b(states[NMT - 2], (NMT - 2) * 2, 2)
        nc.sync.dma_start(out_d[:, :6, :], obuf[:, :6, :])
        postlude_b(st6, NB - 2, 1)
        nc.sync.dma_start(out_d[:, 6:7, :], obuf[:, 6:7, :])
        postlude_b(st7, NB - 1, 1)
        # final piece: block 7 only, split by partition halves onto two
        # queues (sync + scalar) so the packets can spread across engines
        nc.sync.dma_start(out_d[:64, 7:, :], obuf[:64, 7:, :])
        nc.scalar.dma_start(out_d[64:, 7:, :], obuf[64:, 7:, :])

    nc.compile()
    return nc


_CACHE: dict = {}


def _get_compiled(mapping: np.ndarray):
    key = mapping.astype(np.float32).tobytes()
    if _CACHE.get("key") != key:
        col_ids, groups, perm = _structure(mapping)
        nc = _build_program(len(col_ids), groups)
        _CACHE.update(key=key, col_ids=col_ids, groups=groups, perm=perm, nc=nc)
    return _CACHE["nc"], _CACHE["col_ids"], _CACHE["perm"]


def _prep_in_maps(enc_output, feature2phone, col_ids):
    wg = feature2phone.astype(np.float32)[:, col_ids].astype(F8)
    # [H, nnz] -> [128, NH, nnz]
    wk = np.ascontiguousarray(wg.reshape(NH, 128, -1).transpose(1, 0, 2))
    # enc [ROWS, H] -> [128, ROWS, NH]
    e3 = enc_output.astype(F8).reshape(ROWS, NH, 128)
    enck = np.ascontiguousarray(e3.transpose(2, 0, 1))
    in_maps = []
    for c in range(N_CORES):
        in_maps.append({
            "enck": np.ascontiguousarray(enck[:, c * RC:(c + 1) * RC, :]),
            "wk": wk,
        })
    return in_maps


def run_device(enc_output, feature2phone, mapping, trace=False, **kw):
    """Build/compile (cached), run on the 8 cores, return (output, BassKernelResults)."""
    enc_output = np.asarray(enc_output)
    feature2phone = np.asarray(feature2phone)
    mapping = np.asarray(mapping)
    nc, col_ids, perm = _get_compiled(mapping)
    in_maps = _prep_in_maps(enc_output, feature2phone, col_ids)
    res = run_bass_kernel_spmd(
        nc, in_maps, core_ids=list(range(N_CORES)), trace=trace, **kw
    )
    # device out [128, NB, 96] packed -> rows b*128+p
    dev = np.concatenate(
        [res.results[c]["out"].transpose(1, 0, 2).reshape(RC, N_PHONEME)
         for c in range(N_CORES)],
        axis=0,
    )
    out = np.empty_like(dev)
    out[:, perm] = dev
    return out.reshape(B, T, N_PHONEME).astype(np.float32), res


def kernel(enc_output, feature2phone, mapping):
    out, _ = run_device(enc_output, feature2phone, mapping)
    return out


# revision 22
# speedup vs baseline: 1.1295x; 1.1295x over previous
"""Trainium2 Bass kernel for CompositionalPhoneticsModel (segment_reduce).

Computation (reference):
    phone   = einsum('bth,hp->btp', enc_output, feature2phone) / sqrt(H)
    allo    = where(mapping>0, phone[:,:,None,:]*mapping, -inf)   # mapping is 0/1
    phoneme = max(allo, axis=-1)                                  # masked segment max
    out     = log_softmax(phoneme, axis=2)

Device strategy (8 NeuronCores, data-parallel over the B*T=8192 rows):
  * Host gathers feature2phone columns into segment-contiguous order
    (phones in 2 segments get duplicated columns; NNZ ~ 506) and sorts
    segments by length so the per-segment max is a handful of strided DVE
    reduce_max ops.  Host un-permutes the output columns at the end.
  * enc and W ship as fp8 e3m4 (range +-15.5, 4 mantissa bits).  Measured
    absmax/scale error 0.0113 vs the 2e-2 gate, matching the numpy
    simulation bit-for-bit - the PE's single-fp8 path keeps all 4 mantissa
    bits.  The 1/sqrt(H) scale is NOT folded into W (it would push W into
    the e3m4 denormal range); it rides the Exp activation's scale input
    and a fused MULT in the final tensor_scalar.  fp8 halves the DMA bytes
    (loads: 0.97MB/core) so the stream never paces the PE.
  * DMA is bandwidth-bound (~220-290 GB/s/core) once packets (one per
    partition per transfer) are >=1KB, with ~2.4us from program start to
    first byte.  The first matmul gates on W + enc block 0, so W goes
    first, then enc in pieces sized so the stream stays ahead of the PE:
    blocks [0], [1], [2-3], [4-7].  Stores: blocks 0-5 after their subs,
    block 6 alone, block 7 split by partition halves across two queues -
    only ~0.5us of store is exposed after the last compute op.
  * Postlude per 256-row megatile: batched strided segment-max reduces
    (DVE), per-128-row Exp on ScalarE with the row-sum from the activation
    accumulator, one Ln, and a fused (x*scale - lse) DVE tensor_scalar.
    The subs are emitted one megatile LATE: they wait ~1us on ScalarE's
    exp/ln chain and the in-order DVE queue would head-of-line block the
    next megatile's reduces.  The last megatile uses one single-bank PSUM
    tile per 128-row block (a shared tile makes the hazard tracker
    serialize block 7's matmuls behind block 6's reduces) and runs its
    postludes per block, so the post-stream tail is one block's chain.
  * PE warmup matmuls are counted to END right as W + enc block 0 land:
    an idle gap resets the tensor engine's p-state ramp and the next ~14
    real matmuls run ~2x slower.
"""

from contextlib import ExitStack

import numpy as np
import ml_dtypes

import concourse.bass as bass
import concourse.bacc as bacc
import concourse.tile as tile
from concourse import mybir
from concourse.bass_utils import run_bass_kernel_spmd

B, T, H = 8, 1024, 640
N_PHONEME, N_PHONE = 96, 230
N_CORES = 8
ROWS = B * T
RC = ROWS // N_CORES          # rows per core
NH = H // 128                 # contraction chunks
NB = RC // 128                # 128-row blocks per core
NMT = NB // 2                 # megatiles (2 blocks each)
BF16 = ml_dtypes.bfloat16
F8 = ml_dtypes.float8_e3m4
SCALE = float(1.0 / np.sqrt(np.float32(H)))


def _structure(mapping: np.ndarray):
    """Segment-contiguous gather order, grouped by segment length (desc).

    Returns (col_ids, groups, perm):
      col_ids: phone index feeding each device matmul column (len NNZ)
      groups:  list of (L, nL, col_off, out_off) — nL segments of length L
               occupy matmul cols [col_off, col_off+nL*L) and device output
               cols [out_off, out_off+nL)
      perm:    perm[j] = original phoneme id of device output column j
    """
    segs = [np.nonzero(mapping[m] > 0)[0] for m in range(N_PHONEME)]
    assert min(len(s) for s in segs) >= 1
    # pad segment lengths up to even targets (repeating a member doesn't
    # change the max): fewer distinct lengths -> fewer DVE reduce ops.
    # Only worthwhile while the matmul width stays within one PSUM bank.
    padded = []
    for s in segs:
        t = ((len(s) + 1) // 2) * 2
        padded.append(np.concatenate([s, np.full(t - len(s), s[0], s.dtype)]))
    if sum(len(s) for s in padded) <= 512:
        segs = padded
    lengths = np.array([len(s) for s in segs])
    order = np.argsort(-lengths, kind="stable")
    col_ids, groups, perm = [], [], []
    i = 0
    while i < N_PHONEME:
        L = int(lengths[order[i]])
        j = i
        while j < N_PHONEME and lengths[order[j]] == L:
            j += 1
        groups.append((L, j - i, len(col_ids), i))
        for k in range(i, j):
            m = int(order[k])
            col_ids.extend(segs[m].tolist())
            perm.append(m)
        i = j
    return np.array(col_ids, dtype=np.int64), groups, np.array(perm, dtype=np.int64)


def _patch_act_tables():
    """Make Exp and Ln resolve to the same activation-table set.

    bacc's insert_act_table_loads models a single table slot, so a kernel
    alternating Exp/Ln reloads a 1.3us table on every transition.  act_info
    has a joint set ('natural_log_exp_and_others') containing both; keep the
    set list's order/indices intact but strip Exp/Ln from the other sets so
    the pass picks the joint set for both and emits a single load.
    """
    if getattr(bacc, "_act_tables_patched", False):
        return
    from concourse import hw_specs
    orig = hw_specs.get_activation_tables
    act = mybir.ActivationFunctionType

    def patched(module_arch):
        tabs = orig(module_arch)
        joint = [k for k, v in tabs.items() if act.Exp in v and act.Ln in v]
        if not joint:
            return tabs
        j = joint[0]
        return {
            k: (v if k == j else (v - {act.Exp, act.Ln}))
            for k, v in tabs.items()
        }

    bacc.get_activation_tables = patched
    bacc._act_tables_patched = True


def _build_program(nnz: int, groups):
    """Build + compile the per-core Bass program. Returns the Bacc object."""
    _patch_act_tables()
    nc = bacc.Bacc("TRN2", target_bir_lowering=False, debug=False)
    dt = mybir.dt
    act = mybir.ActivationFunctionType
    X = mybir.AxisListType.X
    alu = mybir.AluOpType

    # enc interleaved: [128, RC, NH]; element (p, r, c) = enc[r, c*128+p]
    enck_d = nc.dram_tensor("enck", [128, RC, NH], dt.float8e3, kind="ExternalInput")
    # W interleaved: [128, NH, nnz]; element (p, c, n) = W[c*128+p, n], fp8
    wk_d = nc.dram_tensor("wk", [128, NH, nnz], dt.float8e3, kind="ExternalInput")
    # out packed: [128, NB, 96]; element (p, b, m) = out[b*128+p, m]
    out_d = nc.dram_tensor("out", [128, NB, N_PHONEME], dt.float32, kind="ExternalOutput")

    with ExitStack() as ctx:
        tc = ctx.enter_context(tile.TileContext(nc))
        wpool = ctx.enter_context(tc.tile_pool(name="wpool", bufs=1))
        epool = ctx.enter_context(tc.tile_pool(name="epool", bufs=1))
        # 3 double-bank megatile accumulators + 2 single-bank ones for the
        # last megatile (separate tiles per 128-row block there, so its r0
        # postlude reads never alias r1's accumulation in the hazard
        # tracker - a shared tile serializes the PE behind the DVE)
        ppool = ctx.enter_context(tc.tile_pool(name="ppool", bufs=2, space="PSUM"))
        ppool2 = ctx.enter_context(tc.tile_pool(name="ppool2", bufs=1, space="PSUM"))
        ppool3 = ctx.enter_context(tc.tile_pool(name="ppool3", bufs=2, space="PSUM"))
        spool = ctx.enter_context(tc.tile_pool(name="spool", bufs=3))
        opool = ctx.enter_context(tc.tile_pool(name="opool", bufs=1))

        # Loads, strictly ordered on the Sync queue (the stream is
        # bandwidth-bound, so order = priority): W gates the first matmul,
        # then enc pieces sized to stay ahead of the PE.
        wt = wpool.tile([128, NH, nnz], dt.float8e3)
        et = epool.tile([128, RC, NH], dt.float8e3)
        # the first matmul needs only enc block 0 and W chunks 0-1 (chunk
        # c feeds the c-th accumulation step); later chunks/blocks stream
        # in behind while the PE works
        nc.sync.dma_start(et[:, 0:128, :], enck_d[:, 0:128, :])
        nc.sync.dma_start(wt[:, 0:2, :], wk_d[:, 0:2, :])
        nc.sync.dma_start(wt[:, 2:, :], wk_d[:, 2:, :])
        for lo, hi in ((128, 256), (256, 512), (512, RC)):
            nc.sync.dma_start(et[:, lo:hi, :], enck_d[:, lo:hi, :])

        # PE warmup: dummy matmuls ramp the tensor engine's p-state while
        # the DMAs land.  The operand is the preamble-initialized bf16
        # const (broadcast AP) so no memset gates them - they start at
        # engine-ready (~7.4us) instead of behind the Vector engine.
        # They write the first megatile's PSUM bank; the real
        # accumulation overwrites it.  Count: they must END right as the
        # first matmul's data lands (~10us) - an idle gap resets the
        # ramp (the next ~14 matmuls then run at ~2x duration), overshoot
        # delays the real stream behind the warmup queue.
        wub = nc.const_aps.aps[(dt.bfloat16, 1.0)]
        ps0 = ppool.tile([128, 2, 512], dt.float32, tag="ps")
        for _ in range(5):
            nc.tensor.matmul(ps0[:, 0, :], wub.broadcast_to([128, 128]),
                             wub.broadcast_to([128, 512]), start=True, stop=True)

        obuf = opool.tile([128, NB, N_PHONEME], dt.float32)

        def seg_max(ps, rr, nr, tagn):
            """Segment max of PSUM rows `rr`: one strided DVE reduce per
            segment-length group.  (A pairwise tensor_tensor pre-max would
            halve the reduce work, but the DVE can read only ONE operand
            from PSUM per instruction - walrus NCC_IBVF027.)"""
            pmax = spool.tile([128, nr, N_PHONEME], dt.float32, tag=f"pmax{tagn}")
            for (L, nL, coff, ooff) in groups:
                src = ps[:, rr, coff:coff + nL * L].rearrange(
                    "p r (s l) -> p r s l", l=L
                )
                nc.vector.reduce_max(pmax[:, :, ooff:ooff + nL], src, axis=X)
            return pmax

        def postlude_a(ps, rr, blk, nr):
            """Segment max + exp-sum + Ln for `nr` blocks at `blk`; the
            final subs are emitted LATER (postlude_b) so they queue behind
            the next megatile's reduces: a sub waits on this tile's Ln
            (~1us of ScalarE chain), and the DVE executes in order - subs
            emitted eagerly head-of-line block the next reduces."""
            pmax = seg_max(ps, rr, nr, nr if nr == 2 else blk)
            ex = spool.tile([128, nr, N_PHONEME], dt.float32, tag=f"ex{blk}")
            se = spool.tile([128, nr], dt.float32, tag=f"se{blk}")
            for k in range(nr):
                # exp(scale*x); the row-sum comes free via the activation
                # accumulator (1/sqrt(H) lives here, not in fp8 W)
                nc.scalar.activation(ex[:, k, :], pmax[:, k, :], act.Exp,
                                     scale=SCALE, accum_out=se[:, k:k + 1])
            lse = spool.tile([128, nr], dt.float32, tag=f"lse{blk}")
            nc.scalar.activation(lse[:], se[:], act.Ln)
            return pmax, lse

        def postlude_b(state, blk, nr):
            pmax, lse = state
            for k in range(nr):
                # out = scale*pmax - lse, fused in one DVE op
                nc.vector.tensor_scalar(
                    obuf[:, blk + k, :], pmax[:, k, :],
                    SCALE, lse[:, k:k + 1], op0=alu.mult, op1=alu.subtract,
                )

        def block_matmuls(ps_row, blk, lo=0, hi=None):
            hi = nnz if hi is None else hi
            row0 = blk * 128
            for c in range(NH):
                nc.tensor.matmul(
                    ps_row[:, :hi - lo],
                    et[:, row0:row0 + 128, c],
                    wt[:, c, lo:hi],
                    start=(c == 0),
                    stop=(c == NH - 1),
                )

        states = []
        for mt in range(NMT - 1):
            ps = ps0 if mt == 0 else ppool.tile([128, 2, 512], dt.float32, tag="ps")
            for r in range(2):
                block_matmuls(ps[:, r, :], mt * 2 + r)
            states.append(postlude_a(ps, slice(0, 2), mt * 2, 2))
            if mt > 0:
                postlude_b(states[mt - 1], (mt - 1) * 2, 2)
        # last megatile: one single-bank tile per block so neither block's
        # postlude aliases the other's accumulation in the hazard tracker
        psr0 = ppool2.tile([128, 1, 512], dt.float32, tag="psr")
        psr1 = ppool3.tile([128, 1, 512], dt.float32, tag="psr1")
        block_matmuls(psr0[:, 0, :], NB - 2)
        st6 = postlude_a(psr0, slice(0, 1), NB - 2, 1)
        block_matmuls(psr1[:, 0, :], NB - 1)
        st7 = postlude_a(psr1, slice(0, 1), NB - 1, 1)
        # MT2's subs are deferred past BOTH tail-block reduce sets: on the
        # in-order DVE queue anything emitted between A6 and A7 delays
        # block 7's chain, and the blocks-0-5 store they feed is not the
        # critical DMA (the block-6/7 stores are).
        postlude_b(states[NMT - 2], (NMT - 2) * 2, 2)
        nc.sync.dma_start(out_d[:, :6, :], obuf[:, :6, :])
        postlude_b(st6, NB - 2, 1)
        nc.sync.dma_start(out_d[:, 6:7, :], obuf[:, 6:7, :])
        postlude_b(st7, NB - 1, 1)
        # final piece: block 7 only, split by partition halves onto two
        # queues (sync + scalar) so the packets can spread across engines
        nc.sync.dma_start(out_d[:64, 7:, :], obuf[:64, 7:, :])
        nc.scalar.dma_start(out_d[64:, 7:, :], obuf[64:, 7:, :])

    nc.compile()
    return nc


_CACHE: dict = {}


def _get_compiled(mapping: np.ndarray):
    key = mapping.astype(np.float32).tobytes()
    if _CACHE.get("key") != key:
        col_ids, groups, perm = _structure(mapping)
        nc = _build_program(len(col_ids), groups)
        _CACHE.update(key=key, col_ids=col_ids, groups=groups, perm=perm, nc=nc)
    return _CACHE["nc"], _CACHE["col_ids"], _CACHE["perm"]


def _prep_in_maps(enc_output, feature2phone, col_ids):
    wg = feature2phone.astype(np.float32)[:, col_ids].astype(F8)
    # [H, nnz] -> [128, NH, nnz]
    wk = np.ascontiguousarray(wg.reshape(NH, 128, -1).transpose(1, 0, 2))
    # enc [ROWS, H] -> [128, ROWS, NH]
    e3 = enc_output.astype(F8).reshape(ROWS, NH, 128)
    enck = np.ascontiguousarray(e3.transpose(2, 0, 1))
    in_maps = []
    for c in range(N_CORES):
        in_maps.append({
            "enck": np.ascontiguousarray(enck[:, c * RC:(c + 1) * RC, :]),
            "wk": wk,
        })
    return in_maps


def run_device(enc_output, feature2phone, mapping, trace=False, **kw):
    """Build/compile (cached), run on the 8 cores, return (output, BassKernelResults)."""
    enc_output = np.asarray(enc_output)
    feature2phone = np.asarray(feature2phone)
    mapping = np.asarray(mapping)
    nc, col_ids, perm = _get_compiled(mapping)
    in_maps = _prep_in_maps(enc_output, feature2phone, col_ids)
    res = run_bass_kernel_spmd(
        nc, in_maps, core_ids=list(range(N_CORES)), trace=trace, **kw
    )
    # device out [128, NB, 96] packed -> rows b*128+p
    dev = np.concatenate(
        [res.results[c]["out"].transpose(1, 0, 2).reshape(RC, N_PHONEME)
         for c in range(N_CORES)],
        axis=0,
    )
    out = np.empty_like(dev)
    out[:, perm] = dev
    return out.reshape(B, T, N_PHONEME).astype(np.float32), res


def kernel(enc_output, feature2phone, mapping):
    out, _ = run_device(enc_output, feature2phone, mapping)
    return out


# revision 23
# speedup vs baseline: 1.1488x; 1.0171x over previous
"""Trainium2 Bass kernel for CompositionalPhoneticsModel (segment_reduce).

Computation (reference):
    phone   = einsum('bth,hp->btp', enc_output, feature2phone) / sqrt(H)
    allo    = where(mapping>0, phone[:,:,None,:]*mapping, -inf)   # mapping is 0/1
    phoneme = max(allo, axis=-1)                                  # masked segment max
    out     = log_softmax(phoneme, axis=2)

Device strategy (8 NeuronCores, data-parallel over the B*T=8192 rows):
  * Host gathers feature2phone columns into segment-contiguous order
    (phones in 2 segments get duplicated columns; NNZ ~ 506) and sorts
    segments by length so the per-segment max is a handful of strided DVE
    reduce_max ops.  Host un-permutes the output columns at the end.
  * enc and W ship as fp8 e3m4 (range +-15.5, 4 mantissa bits).  Measured
    absmax/scale error 0.0113 vs the 2e-2 gate, matching the numpy
    simulation bit-for-bit - the PE's single-fp8 path keeps all 4 mantissa
    bits.  The 1/sqrt(H) scale is NOT folded into W (it would push W into
    the e3m4 denormal range); it rides the Exp activation's scale input
    and a fused MULT in the final tensor_scalar.  fp8 halves the DMA bytes
    (loads: 0.97MB/core) so the stream never paces the PE.
  * DMA is bandwidth-bound (~220-290 GB/s/core) once packets (one per
    partition per transfer) are >=1KB, with ~2.4us from program start to
    first byte.  The first matmul gates on W + enc block 0, so W goes
    first, then enc in pieces sized so the stream stays ahead of the PE:
    blocks [0], [1], [2-3], [4-7].  Stores: blocks 0-5 after their subs,
    block 6 alone, block 7 split by partition halves across two queues -
    only ~0.5us of store is exposed after the last compute op.
  * Postlude per 256-row megatile: batched strided segment-max reduces
    (DVE), per-128-row Exp on ScalarE with the row-sum from the activation
    accumulator, one Ln, and a fused (x*scale - lse) DVE tensor_scalar.
    The subs are emitted one megatile LATE: they wait ~1us on ScalarE's
    exp/ln chain and the in-order DVE queue would head-of-line block the
    next megatile's reduces.  The last megatile uses one single-bank PSUM
    tile per 128-row block (a shared tile makes the hazard tracker
    serialize block 7's matmuls behind block 6's reduces) and runs its
    postludes per block, so the post-stream tail is one block's chain.
  * PE warmup matmuls are counted to END right as W + enc block 0 land:
    an idle gap resets the tensor engine's p-state ramp and the next ~14
    real matmuls run ~2x slower.
"""

from contextlib import ExitStack

import numpy as np
import ml_dtypes

import concourse.bass as bass
import concourse.bacc as bacc
import concourse.tile as tile
from concourse import mybir
from concourse.bass_utils import run_bass_kernel_spmd

B, T, H = 8, 1024, 640
N_PHONEME, N_PHONE = 96, 230
N_CORES = 8
ROWS = B * T
RC = ROWS // N_CORES          # rows per core
NH = H // 128                 # contraction chunks
NB = RC // 128                # 128-row blocks per core
NMT = NB // 2                 # megatiles (2 blocks each)
BF16 = ml_dtypes.bfloat16
F8 = ml_dtypes.float8_e3m4
SCALE = float(1.0 / np.sqrt(np.float32(H)))


def _structure(mapping: np.ndarray):
    """Segment-contiguous gather order, grouped by segment length (desc).

    Returns (col_ids, groups, perm):
      col_ids: phone index feeding each device matmul column (len NNZ)
      groups:  list of (L, nL, col_off, out_off) — nL segments of length L
               occupy matmul cols [col_off, col_off+nL*L) and device output
               cols [out_off, out_off+nL)
      perm:    perm[j] = original phoneme id of device output column j
    """
    segs = [np.nonzero(mapping[m] > 0)[0] for m in range(N_PHONEME)]
    assert min(len(s) for s in segs) >= 1
    # pad segment lengths up to even targets (repeating a member doesn't
    # change the max): fewer distinct lengths -> fewer DVE reduce ops.
    # Only worthwhile while the matmul width stays within one PSUM bank.
    padded = []
    for s in segs:
        t = ((len(s) + 1) // 2) * 2
        padded.append(np.concatenate([s, np.full(t - len(s), s[0], s.dtype)]))
    if sum(len(s) for s in padded) <= 512:
        segs = padded
    lengths = np.array([len(s) for s in segs])
    order = np.argsort(-lengths, kind="stable")
    col_ids, groups, perm = [], [], []
    i = 0
    while i < N_PHONEME:
        L = int(lengths[order[i]])
        j = i
        while j < N_PHONEME and lengths[order[j]] == L:
            j += 1
        groups.append((L, j - i, len(col_ids), i))
        for k in range(i, j):
            m = int(order[k])
            col_ids.extend(segs[m].tolist())
            perm.append(m)
        i = j
    return np.array(col_ids, dtype=np.int64), groups, np.array(perm, dtype=np.int64)


def _patch_act_tables():
    """Make Exp and Ln resolve to the same activation-table set.

    bacc's insert_act_table_loads models a single table slot, so a kernel
    alternating Exp/Ln reloads a 1.3us table on every transition.  act_info
    has a joint set ('natural_log_exp_and_others') containing both; keep the
    set list's order/indices intact but strip Exp/Ln from the other sets so
    the pass picks the joint set for both and emits a single load.
    """
    if getattr(bacc, "_act_tables_patched", False):
        return
    from concourse import hw_specs
    orig = hw_specs.get_activation_tables
    act = mybir.ActivationFunctionType

    def patched(module_arch):
        tabs = orig(module_arch)
        joint = [k for k, v in tabs.items() if act.Exp in v and act.Ln in v]
        if not joint:
            return tabs
        j = joint[0]
        return {
            k: (v if k == j else (v - {act.Exp, act.Ln}))
            for k, v in tabs.items()
        }

    bacc.get_activation_tables = patched
    bacc._act_tables_patched = True


def _build_program(nnz: int, groups):
    """Build + compile the per-core Bass program. Returns the Bacc object."""
    _patch_act_tables()
    nc = bacc.Bacc("TRN2", target_bir_lowering=False, debug=False)
    dt = mybir.dt
    act = mybir.ActivationFunctionType
    X = mybir.AxisListType.X
    alu = mybir.AluOpType

    # enc interleaved: [128, RC, NH]; element (p, r, c) = enc[r, c*128+p]
    enck_d = nc.dram_tensor("enck", [128, RC, NH], dt.float8e3, kind="ExternalInput")
    # W interleaved: [128, NH, nnz]; element (p, c, n) = W[c*128+p, n], fp8
    wk_d = nc.dram_tensor("wk", [128, NH, nnz], dt.float8e3, kind="ExternalInput")
    # out packed: [128, NB, 96]; element (p, b, m) = out[b*128+p, m]
    out_d = nc.dram_tensor("out", [128, NB, N_PHONEME], dt.float32, kind="ExternalOutput")

    with ExitStack() as ctx:
        tc = ctx.enter_context(tile.TileContext(nc))
        wpool = ctx.enter_context(tc.tile_pool(name="wpool", bufs=1))
        epool = ctx.enter_context(tc.tile_pool(name="epool", bufs=1))
        # 3 double-bank megatile accumulators + 2 single-bank ones for the
        # last megatile (separate tiles per 128-row block there, so its r0
        # postlude reads never alias r1's accumulation in the hazard
        # tracker - a shared tile serializes the PE behind the DVE)
        ppool = ctx.enter_context(tc.tile_pool(name="ppool", bufs=2, space="PSUM"))
        ppool2 = ctx.enter_context(tc.tile_pool(name="ppool2", bufs=1, space="PSUM"))
        ppool3 = ctx.enter_context(tc.tile_pool(name="ppool3", bufs=2, space="PSUM"))
        spool = ctx.enter_context(tc.tile_pool(name="spool", bufs=3))
        opool = ctx.enter_context(tc.tile_pool(name="opool", bufs=1))

        # Loads, strictly ordered on the Sync queue (the stream is
        # bandwidth-bound, so order = priority): W gates the first matmul,
        # then enc pieces sized to stay ahead of the PE.
        wt = wpool.tile([128, NH, nnz], dt.float8e3)
        et = epool.tile([128, RC, NH], dt.float8e3)
        # the first matmul needs only enc block 0 and W chunks 0-1 (chunk
        # c feeds the c-th accumulation step); later chunks/blocks stream
        # in behind while the PE works
        nc.sync.dma_start(et[:, 0:128, :], enck_d[:, 0:128, :])
        nc.sync.dma_start(wt[:, 0:2, :], wk_d[:, 0:2, :])
        nc.sync.dma_start(wt[:, 2:, :], wk_d[:, 2:, :])
        for lo, hi in ((128, 256), (256, 512), (512, RC)):
            nc.sync.dma_start(et[:, lo:hi, :], enck_d[:, lo:hi, :])

        # PE warmup: dummy matmuls ramp the tensor engine's p-state while
        # the DMAs land.  The operand is the preamble-initialized bf16
        # const (broadcast AP) so no memset gates them - they start at
        # engine-ready (~7.4us) instead of behind the Vector engine.
        # They write the first megatile's PSUM bank; the real
        # accumulation overwrites it.  Count: they must END right as the
        # first matmul's data lands (~10us) - an idle gap resets the
        # ramp (the next ~14 matmuls then run at ~2x duration), overshoot
        # delays the real stream behind the warmup queue.
        wub = nc.const_aps.aps[(dt.bfloat16, 1.0)]
        ps0 = ppool.tile([128, 2, 512], dt.float32, tag="ps")
        for _ in range(7):
            nc.tensor.matmul(ps0[:, 0, :], wub.broadcast_to([128, 128]),
                             wub.broadcast_to([128, 512]), start=True, stop=True)

        obuf = opool.tile([128, NB, N_PHONEME], dt.float32)

        def seg_max(ps, rr, nr, tagn):
            """Segment max of PSUM rows `rr`: one strided DVE reduce per
            segment-length group.  (A pairwise tensor_tensor pre-max would
            halve the reduce work, but the DVE can read only ONE operand
            from PSUM per instruction - walrus NCC_IBVF027.)"""
            pmax = spool.tile([128, nr, N_PHONEME], dt.float32, tag=f"pmax{tagn}")
            for (L, nL, coff, ooff) in groups:
                src = ps[:, rr, coff:coff + nL * L].rearrange(
                    "p r (s l) -> p r s l", l=L
                )
                nc.vector.reduce_max(pmax[:, :, ooff:ooff + nL], src, axis=X)
            return pmax

        def postlude_a(ps, rr, blk, nr):
            """Segment max + exp-sum + Ln for `nr` blocks at `blk`; the
            final subs are emitted LATER (postlude_b) so they queue behind
            the next megatile's reduces: a sub waits on this tile's Ln
            (~1us of ScalarE chain), and the DVE executes in order - subs
            emitted eagerly head-of-line block the next reduces."""
            pmax = seg_max(ps, rr, nr, nr if nr == 2 else blk)
            ex = spool.tile([128, nr, N_PHONEME], dt.float32, tag=f"ex{blk}")
            se = spool.tile([128, nr], dt.float32, tag=f"se{blk}")
            for k in range(nr):
                # exp(scale*x); the row-sum comes free via the activation
                # accumulator (1/sqrt(H) lives here, not in fp8 W)
                nc.scalar.activation(ex[:, k, :], pmax[:, k, :], act.Exp,
                                     scale=SCALE, accum_out=se[:, k:k + 1])
            lse = spool.tile([128, nr], dt.float32, tag=f"lse{blk}")
            nc.scalar.activation(lse[:], se[:], act.Ln)
            return pmax, lse

        def postlude_b(state, blk, nr):
            pmax, lse = state
            for k in range(nr):
                # out = scale*pmax - lse, fused in one DVE op
                nc.vector.tensor_scalar(
                    obuf[:, blk + k, :], pmax[:, k, :],
                    SCALE, lse[:, k:k + 1], op0=alu.mult, op1=alu.subtract,
                )

        def block_matmuls(ps_row, blk, lo=0, hi=None):
            hi = nnz if hi is None else hi
            row0 = blk * 128
            for c in range(NH):
                nc.tensor.matmul(
                    ps_row[:, :hi - lo],
                    et[:, row0:row0 + 128, c],
                    wt[:, c, lo:hi],
                    start=(c == 0),
                    stop=(c == NH - 1),
                )

        states = []
        for mt in range(NMT - 1):
            ps = ps0 if mt == 0 else ppool.tile([128, 2, 512], dt.float32, tag="ps")
            for r in range(2):
                block_matmuls(ps[:, r, :], mt * 2 + r)
            states.append(postlude_a(ps, slice(0, 2), mt * 2, 2))
            if mt > 0:
                postlude_b(states[mt - 1], (mt - 1) * 2, 2)
        # last megatile: one single-bank tile per block so neither block's
        # postlude aliases the other's accumulation in the hazard tracker
        psr0 = ppool2.tile([128, 1, 512], dt.float32, tag="psr")
        psr1 = ppool3.tile([128, 1, 512], dt.float32, tag="psr1")
        block_matmuls(psr0[:, 0, :], NB - 2)
        st6 = postlude_a(psr0, slice(0, 1), NB - 2, 1)
        block_matmuls(psr1[:, 0, :], NB - 1)
        st7 = postlude_a(psr1, slice(0, 1), NB - 1, 1)
        # MT2's subs are deferred past BOTH tail-block reduce sets: on the
        # in-order DVE queue anything emitted between A6 and A7 delays
        # block 7's chain, and the blocks-0-5 store they feed is not the
        # critical DMA (the block-6/7 stores are).
        postlude_b(states[NMT - 2], (NMT - 2) * 2, 2)
        nc.sync.dma_start(out_d[:, :6, :], obuf[:, :6, :])
        postlude_b(st6, NB - 2, 1)
        nc.sync.dma_start(out_d[:, 6:7, :], obuf[:, 6:7, :])
        postlude_b(st7, NB - 1, 1)
        # final piece: block 7 only, split by partition halves onto two
        # queues (sync + scalar) so the packets can spread across engines
        nc.sync.dma_start(out_d[:64, 7:, :], obuf[:64, 7:, :])
        nc.scalar.dma_start(out_d[64:, 7:, :], obuf[64:, 7:, :])

    nc.compile()
    return nc


_CACHE: dict = {}


def _get_compiled(mapping: np.ndarray):
    key = mapping.astype(np.float32).tobytes()
    if _CACHE.get("key") != key:
        col_ids, groups, perm = _structure(mapping)
        nc = _build_program(len(col_ids), groups)
        _CACHE.update(key=key, col_ids=col_ids, groups=groups, perm=perm, nc=nc)
    return _CACHE["nc"], _CACHE["col_ids"], _CACHE["perm"]


def _prep_in_maps(enc_output, feature2phone, col_ids):
    wg = feature2phone.astype(np.float32)[:, col_ids].astype(F8)
    # [H, nnz] -> [128, NH, nnz]
    wk = np.ascontiguousarray(wg.reshape(NH, 128, -1).transpose(1, 0, 2))
    # enc [ROWS, H] -> [128, ROWS, NH]
    e3 = enc_output.astype(F8).reshape(ROWS, NH, 128)
    enck = np.ascontiguousarray(e3.transpose(2, 0, 1))
    in_maps = []
    for c in range(N_CORES):
        in_maps.append({
            "enck": np.ascontiguousarray(enck[:, c * RC:(c + 1) * RC, :]),
            "wk": wk,
        })
    return in_maps


def run_device(enc_output, feature2phone, mapping, trace=False, **kw):
    """Build/compile (cached), run on the 8 cores, return (output, BassKernelResults)."""
    enc_output = np.asarray(enc_output)
    feature2phone = np.asarray(feature2phone)
    mapping = np.asarray(mapping)
    nc, col_ids, perm = _get_compiled(mapping)
    in_maps = _prep_in_maps(enc_output, feature2phone, col_ids)
    res = run_bass_kernel_spmd(
        nc, in_maps, core_ids=list(range(N_CORES)), trace=trace, **kw
    )
    # device out [128, NB, 96] packed -> rows b*128+p
    dev = np.concatenate(
        [res.results[c]["out"].transpose(1, 0, 2).reshape(RC, N_PHONEME)
         for c in range(N_CORES)],
        axis=0,
    )
    out = np.empty_like(dev)
    out[:, perm] = dev
    return out.reshape(B, T, N_PHONEME).astype(np.float32), res


def kernel(enc_output, feature2phone, mapping):
    out, _ = run_device(enc_output, feature2phone, mapping)
    return out
